# revision 14
# baseline (speedup 1.0000x reference)
"""Trainium2 Bass kernel: batch row-sharded grouped GEMM (MoE routing).

Contract: kernel(x, weight, num_inputs_per_group) takes FULL inputs
  x (32768, 2048) f32, weight (16, 2048, 2048) f32, num_inputs_per_group (16,) i32
and returns the FULL output (32768, 2048) f32, where token row i is multiplied
by weight[seg[i]] with seg = repeat(arange(16), num, total_repeat_length=32768)
(contiguous groups).

Distribution strategy (no collectives needed): tokens are split contiguously;
each of the 8 cores gets 4096 tokens plus the weights of the experts those
tokens use (expert/token parallelism — sanctioned by the sharding hint since
E=16 >= 8). Each core computes its grouped GEMM locally and the host
concatenates the per-core outputs.

Fast path (uniform 2048-token groups, the shipped distribution): one level of
STRASSEN per (2048 x 2048) @ (2048 x 2048) expert GEMM. Each run (one expert,
2048 tokens) is split into 2x2 blocks of 1024: 7 products instead of 8 cuts
the PE matmul floor by 12.5% (442us -> 387us across 8 cores); the extra
elementwise adds ride on the otherwise-idle Vector/Scalar/GpSimd engines,
fully hidden under PE compute.

Device schedule per run — multiply-index OUTER (stage), token-block-pair
inner, so each 2 MiB weight tensor is consumed over a full 27.6us stage
(76 GB/s demand; no HBM ramp spike) and dies at stage end (wpool bufs=3):
  stage: 0:M4=A22*T4  1:M3=A11*T3  2:M5=S3*B22  3:M2=S2*B11
         4:M1=S1*T1   5:M7=S5*T7   6:M6=S4*T6
  (T1=B11+B22, T3=B12-B22, T4=B21-B11, T6=B11+B12, T7=B21+B22 precomputed
   on HOST in fp32, shipped bf16 — 7 weight tensors of [1024,1024] per run.)
Pairing: token block p (rows p*128 of the run's first 1024) pairs with block
p+8 (second 1024); S-prep sums (GpSimd, bf16) and partial combines (Vector,
reading PSUM fp32 directly, bf16 partials) follow Strassen:
  s4=cp(M4); m3cp=cp(M3); C12=m3cp+M5, c11p=s4-M5; C21=s4+M2, c22b=m3cp-M2;
  t1=c11p+M1, t22=c22b+M1; C11=t1+M7; C22=t22+M6.
Copies on Scalar (ACT), preps on GpSimd, combines on Vector — each <=80% busy
per stage. Outputs drain as bf16 (host upcasts); rel err ~5.9e-3 (gate 2e-2).
Rings: sync = weights (28 MiB), scalar = x in + out (24 MiB) — no saturated
queue (baseline's scalar ring carried 65 MiB at ~150 GB/s limit).
Ramp: first stage needs only x's A22 quarter (0.125 MiB, first scalar job)
plus T4's first k-quarter (0.5 MiB, first sync job); ~14 throwaway bf16
warmup matmuls bridge the ~8.2us SPMD preamble into first real work while
holding the PE HAM clock at 8/8.
"""

import sys

sys.path.insert(0, "/opt/trn_rl_repo")

import numpy as np
import ml_dtypes

import concourse.bacc as bacc
import concourse.mybir as mybir
from concourse.bass_utils import run_bass_kernel_spmd
from concourse.tile import TileContext
from concourse.tile_rust import add_dep_helper

BF16 = ml_dtypes.bfloat16

N_TOK, D_IN, D_OUT, N_EXP = 32768, 2048, 2048, 16
NCORES = 8
PB = 128  # token block = PSUM partition count

# ---- Strassen fast-path geometry ----
RUNS = 2          # experts (runs) per core
RUN_T = 2048      # tokens per run
NPAIR = 8         # 128-token block pairs per run (block p with block p+8)
KO = 8            # k-outer tiles per 1024-half (8 x 128)
NH = 512          # PSUM tile width (one bank)
N_WARM = 12

# Introspection hooks for test.py (harness just calls kernel()).
TRACE = False
LAST_RESULTS = None


def _seg_from_groups(num):
    """Replicate jnp.repeat(arange(E), num, total_repeat_length=N) semantics."""
    num = np.asarray(num, dtype=np.int64)
    reps = np.repeat(np.arange(N_EXP, dtype=np.int32), np.maximum(num, 0))
    if len(reps) >= N_TOK:
        return reps[:N_TOK]
    pad = reps[-1] if len(reps) else np.int32(0)
    return np.concatenate([reps, np.full(N_TOK - len(reps), pad, np.int32)])


# ===========================================================================
# Strassen fast path (uniform 2048-token groups -> 2 experts per core)
# ===========================================================================

# stage -> (host weight matrix index, stationary kind)
#   stationary kinds: 'A22', 'A11' read x tile slices directly;
#   'S3','S2','S1','S5','S4' are GpSimd-prepped sums.
_STAGES = [
    ("A22", None),  # M4 = A22 @ T4
    ("A11", None),  # M3 = A11 @ T3
    ("S3", "add_00_10"),   # M5 = (A11+A12) @ B22
    ("S2", "add_01_11"),   # M2 = (A21+A22) @ B11
    ("S1", "add_00_11"),   # M1 = (A11+A22) @ T1
    ("S5", "sub_10_11"),   # M7 = (A12-A22) @ T7
    ("S4", "sub_01_00"),   # M6 = (A21-A11) @ T6
]
# x tile layout [128(d%128), 16(d//128), 256(tok: r1 128 | r2 128)]:
#   A11 = [:, 0:8, 0:128]    A12 = [:, 8:16, 0:128]
#   A21 = [:, 0:8, 128:256]  A22 = [:, 8:16, 128:256]
_XSLICE = {
    "A11": (0, 0), "A12": (8, 0), "A21": (0, 128), "A22": (8, 128),
}
_PREP = {  # S kind -> (lhs slice, rhs slice, is_sub)
    "S3": ("A11", "A12", False),
    "S2": ("A21", "A22", False),
    "S1": ("A11", "A22", False),
    "S5": ("A12", "A22", True),
    "S4": ("A21", "A11", True),
}


def _build_nc_strassen():
    f32 = mybir.dt.float32
    bf16 = mybir.dt.bfloat16

    nc = bacc.Bacc("TRN2", target_bir_lowering=False, debug=False, num_devices=NCORES)
    xh = nc.dram_tensor("xh", [RUNS, NPAIR, PB, 2 * KO, 2 * PB], bf16,
                        kind="ExternalInput")
    wh = nc.dram_tensor("wh", [RUNS, 7, PB, KO, 2 * NH], bf16,
                        kind="ExternalInput")
    out = nc.dram_tensor("out", [RUNS * RUN_T, D_OUT], bf16, kind="ExternalOutput")

    with TileContext(nc) as tc:
        with (
            tc.tile_pool(name="wpool", bufs=4) as wpool,
            tc.tile_pool(name="xpool", bufs=9) as xpool,
            tc.tile_pool(name="spool", bufs=5) as spool,
            tc.tile_pool(name="ppool", bufs=24) as ppool,
            tc.tile_pool(name="opool", bufs=5) as opool,
            tc.tile_pool(name="pspool", bufs=8, space="PSUM") as pspool,
        ):
            # ---- weight streams: all 14 tensors upfront on the sync ring.
            # wpool bufs=3 + engine FIFO = natural just-in-time pacing with
            # ~2 stages of lookahead. k-quarter split gives the ramp
            # fine-grained deps. The very first tensor (run0 T4) is split
            # k-eighth on sync plus k-quarters on the otherwise-idle gpsimd
            # ring so the ramp's first bytes land at ~2x bandwidth (these
            # have no pool-reuse waits, so they cannot block either FIFO).
            wtiles = [[None] * 7 for _ in range(RUNS)]
            for r in range(RUNS):
                for si in range(7):
                    t = wpool.tile([PB, KO, 2 * NH], bf16, name=f"w_{r}_{si}",
                                   tag="w")
                    if r == 0 and si == 0:
                        # First tensor feeds the ramp: the consumption order
                        # is (nh0: k0..7), (nh1: k0..7), so deliver nh0 via
                        # two queues in parallel, then nh1 the same way.
                        nc.sync.dma_start(out=t[:, 0:2, :NH], in_=wh[r, si][:, 0:2, :NH])
                        nc.sync.dma_start(out=t[:, 2:4, :NH], in_=wh[r, si][:, 2:4, :NH])
                        nc.gpsimd.dma_start(out=t[:, 4:6, :NH], in_=wh[r, si][:, 4:6, :NH])
                        nc.gpsimd.dma_start(out=t[:, 6:8, :NH], in_=wh[r, si][:, 6:8, :NH])
                        nc.sync.dma_start(out=t[:, 0:4, NH:], in_=wh[r, si][:, 0:4, NH:])
                        nc.gpsimd.dma_start(out=t[:, 4:8, NH:], in_=wh[r, si][:, 4:8, NH:])
                    else:
                        for q in range(4):
                            nc.sync.dma_start(
                                out=t[:, 2 * q:2 * q + 2, :],
                                in_=wh[r, si][:, 2 * q:2 * q + 2, :],
                            )
                    wtiles[r][si] = t

            # ---- x streams on the scalar ring: per run, first the A22
            # quarters of every pair (stage 0's only need), then the rest.
            # Run 0's jobs are emitted up front; run r+1's are emitted at
            # run r's stage-5 boundary (see main loop) so their xpool-reuse
            # waits sit in the scalar FIFO only behind out-DMAs whose data
            # is ready well before those waits clear.
            xtiles = [[None] * NPAIR for _ in range(RUNS)]

            def emit_x(r):
                for p in range(NPAIR):
                    t = xpool.tile([PB, 2 * KO, 2 * PB], bf16,
                                   name=f"x_{r}_{p}", tag="x")
                    nc.scalar.dma_start(out=t[:, KO:, PB:], in_=xh[r, p][:, KO:, PB:])
                    xtiles[r][p] = t
                for p in range(NPAIR):
                    t = xtiles[r][p]
                    nc.scalar.dma_start(out=t[:, :KO, :], in_=xh[r, p][:, :KO, :])
                    nc.scalar.dma_start(out=t[:, KO:, :PB], in_=xh[r, p][:, KO:, :PB])

            emit_x(0)

            # ---- PE warmup: throwaway bf16 matmuls bridge the SPMD preamble
            # until the first weight quarter lands, holding HAM at 8/8.
            wsrc = spool.tile([PB, 2 * PB], bf16, name="warm_src", tag="warm")
            nc.vector.memset(wsrc, 0.0)
            wps = pspool.tile([PB, NH], f32, name="warm_ps", tag="ps")
            for _ in range(N_WARM):
                nc.tensor.matmul(wps[:, :2 * PB], wsrc[:, :PB], wsrc,
                                 start=True, stop=True)

            def xsl(xt, kind, k=None):
                ko0, t0 = _XSLICE[kind]
                if k is None:
                    return xt[:, ko0:ko0 + KO, t0:t0 + PB]
                return xt[:, ko0 + k, t0:t0 + PB]

            for r in range(RUNS):
                rb = r * RUN_T
                # per-pair partials (bf16 [128, 1024]) living across stages
                s4 = [None] * NPAIR
                m3cp = [None] * NPAIR
                c11p = [None] * NPAIR
                c22b = [None] * NPAIR
                t1 = [None] * NPAIR
                t22 = [None] * NPAIR
                for si, (stat, prep) in enumerate(_STAGES):
                    if si == 5 and r + 1 < RUNS:
                        emit_x(r + 1)
                    wt = wtiles[r][si]
                    # S-preps for this stage (GpSimd, bf16): emitted ahead of
                    # the consuming pair's matmuls; gpsimd runs ~1 pair ahead.
                    stiles = [None] * NPAIR
                    if prep is not None:
                        a, b, is_sub = _PREP[stat]
                        for p in range(NPAIR):
                            st = spool.tile([PB, KO, PB], bf16,
                                            name=f"s_{r}_{si}_{p}", tag="s")
                            xt = xtiles[r][p]
                            if is_sub:
                                nc.gpsimd.tensor_sub(st, xsl(xt, a), xsl(xt, b))
                            else:
                                nc.gpsimd.tensor_add(st, xsl(xt, a), xsl(xt, b))
                            stiles[p] = st
                    for p in range(NPAIR):
                        xt = xtiles[r][p]
                        ps = []
                        for nh in range(2):
                            pst = pspool.tile([PB, NH], f32, name="ps", tag="ps")
                            for k in range(KO):
                                if prep is None:
                                    lhsT = xsl(xt, stat, k)
                                else:
                                    lhsT = stiles[p][:, k, :]
                                nc.tensor.matmul(
                                    pst, lhsT, wt[:, k, nh * NH:(nh + 1) * NH],
                                    start=(k == 0), stop=(k == KO - 1),
                                )
                            ps.append(pst)
                        r1 = rb + p * PB
                        r2 = rb + RUN_T // 2 + p * PB
                        if si == 0:  # M4 -> s4 = cp(M4)   (vector is idle here)
                            s4[p] = ppool.tile([PB, 2 * NH], bf16,
                                               name=f"s4_{r}_{p}", tag="pp")
                            for h in range(2):
                                nc.vector.tensor_copy(
                                    out=s4[p][:, h * NH:(h + 1) * NH], in_=ps[h])
                        elif si == 1:  # M3 -> m3cp = cp(M3)
                            m3cp[p] = ppool.tile([PB, 2 * NH], bf16,
                                                 name=f"m3_{r}_{p}", tag="pp")
                            for h in range(2):
                                nc.vector.tensor_copy(
                                    out=m3cp[p][:, h * NH:(h + 1) * NH], in_=ps[h])
                        elif si == 2:  # M5: C12 = m3cp + M5 ; c11p = s4 - M5
                            ot = opool.tile([PB, 2 * NH], bf16,
                                            name=f"c12_{r}_{p}", tag="o")
                            c11p[p] = ppool.tile([PB, 2 * NH], bf16,
                                                 name=f"c11p_{r}_{p}", tag="pp")
                            for h in range(2):
                                hs = slice(h * NH, (h + 1) * NH)
                                nc.vector.tensor_add(ot[:, hs], m3cp[p][:, hs], ps[h])
                                nc.vector.tensor_sub(c11p[p][:, hs], s4[p][:, hs], ps[h])
                            nc.scalar.dma_start(
                                out=out[r1:r1 + PB, D_OUT // 2:], in_=ot)
                        elif si == 3:  # M2: C21 = s4 + M2 ; c22b = m3cp - M2
                            ot = opool.tile([PB, 2 * NH], bf16,
                                            name=f"c21_{r}_{p}", tag="o")
                            c22b[p] = ppool.tile([PB, 2 * NH], bf16,
                                                 name=f"c22b_{r}_{p}", tag="pp")
                            for h in range(2):
                                hs = slice(h * NH, (h + 1) * NH)
                                nc.vector.tensor_add(ot[:, hs], s4[p][:, hs], ps[h])
                                nc.vector.tensor_sub(c22b[p][:, hs], m3cp[p][:, hs], ps[h])
                            nc.scalar.dma_start(
                                out=out[r2:r2 + PB, :D_OUT // 2], in_=ot)
                        elif si == 4:  # M1: t1 = c11p + M1 ; t22 = c22b + M1
                            t1[p] = ppool.tile([PB, 2 * NH], bf16,
                                               name=f"t1_{r}_{p}", tag="pp")
                            t22[p] = ppool.tile([PB, 2 * NH], bf16,
                                                name=f"t22_{r}_{p}", tag="pp")
                            for h in range(2):
                                hs = slice(h * NH, (h + 1) * NH)
                                nc.vector.tensor_add(t1[p][:, hs], c11p[p][:, hs], ps[h])
                                nc.vector.tensor_add(t22[p][:, hs], c22b[p][:, hs], ps[h])
                        elif si == 5:  # M7: C11 = t1 + M7
                            ot = opool.tile([PB, 2 * NH], bf16,
                                            name=f"c11_{r}_{p}", tag="o")
                            for h in range(2):
                                hs = slice(h * NH, (h + 1) * NH)
                                nc.vector.tensor_add(ot[:, hs], t1[p][:, hs], ps[h])
                            nc.scalar.dma_start(
                                out=out[r1:r1 + PB, :D_OUT // 2], in_=ot)
                        else:  # si == 6, M6: C22 = t22 + M6
                            ot = opool.tile([PB, 2 * NH], bf16,
                                            name=f"c22_{r}_{p}", tag="o")
                            last = r == RUNS - 1 and p == NPAIR - 1
                            for h in range(2):
                                hs = slice(h * NH, (h + 1) * NH)
                                nc.vector.tensor_add(ot[:, hs], t22[p][:, hs], ps[h])
                                if last:
                                    # drain the kernel's final tile per half so
                                    # the last DMA overlaps the last vec op
                                    nc.scalar.dma_start(
                                        out=out[r2:r2 + PB,
                                                D_OUT // 2 + h * NH:
                                                D_OUT // 2 + (h + 1) * NH],
                                        in_=ot[:, hs])
                            if not last:
                                nc.scalar.dma_start(
                                    out=out[r2:r2 + PB, D_OUT // 2:], in_=ot)
    nc.compile()
    return nc


def _host_pack_x_strassen(xc):
    """Pack a core's 4096 tokens into bf16 pair tiles [2, 8, 128, 16, 256]."""
    x16 = np.asarray(xc, dtype=BF16)
    xhp = np.empty((RUNS, NPAIR, PB, 2 * KO, 2 * PB), dtype=BF16)
    for r in range(RUNS):
        xr = x16[r * RUN_T:(r + 1) * RUN_T]
        for p in range(NPAIR):
            r1 = xr[p * PB:(p + 1) * PB]
            r2 = xr[RUN_T // 2 + p * PB:RUN_T // 2 + (p + 1) * PB]
            blk = np.concatenate([r1, r2], axis=0)  # [256, 2048]
            # (tok, d) -> (d%128, d//128, tok)
            xhp[r, p] = blk.reshape(2 * PB, 2 * KO, PB).transpose(2, 1, 0)
    return np.ascontiguousarray(xhp)


def _host_pack_w_strassen(experts_w):
    """Pack per-run expert weights [2, 2048, 2048] f32 into the 7 Strassen
    weight tensors per run, bf16 tiles [2, 7, 128, 8, 1024] (stage order)."""
    whp = np.empty((RUNS, 7, PB, KO, 2 * NH), dtype=BF16)
    for r in range(RUNS):
        we = experts_w[r]
        h = D_IN // 2
        B11, B12 = we[:h, :h], we[:h, h:]
        B21, B22 = we[h:, :h], we[h:, h:]
        mats = [B21 - B11, B12 - B22, B22, B11, B11 + B22, B21 + B22, B11 + B12]
        for si, m in enumerate(mats):
            # (k, n) -> (k%128, k//128, n)
            whp[r, si] = m.astype(BF16).reshape(KO, PB, 2 * NH).transpose(1, 0, 2)
    return np.ascontiguousarray(whp)


def _kernel_strassen(x, weight):
    global LAST_RESULTS
    in_maps = []
    for c in range(NCORES):
        xc = x[c * RUNS * RUN_T:(c + 1) * RUNS * RUN_T]
        ew = weight[[2 * c, 2 * c + 1]]
        in_maps.append({
            "xh": _host_pack_x_strassen(xc),
            "wh": _host_pack_w_strassen(ew),
        })
    nc = _build_nc_strassen()
    res = run_bass_kernel_spmd(nc, in_maps, core_ids=list(range(NCORES)),
                               trace=TRACE)
    LAST_RESULTS = res
    outs = [np.asarray(res.results[c]["out"], dtype=np.float32)
            for c in range(NCORES)]
    return np.concatenate(outs, axis=0)


# ===========================================================================
# General fallback path (previous-generation kernel, handles any grouping)
# ===========================================================================

NT = 512  # matmul moving free dim = one fp32 PSUM bank
KT = D_IN // PB  # 16 k-tiles
NTILES = D_OUT // NT  # 4 output column sets
MG_BLOCKS = 2  # token blocks per x group tile
MGT = MG_BLOCKS * PB  # tokens per group tile
DEFER = 2  # chunks of run 0 whose n1 work is deferred past chunk DEFER's


def _run_groups(runs):
    groups = []  # (run_idx, g_blocks)
    for ri, (_, nb) in enumerate(runs):
        last_run = ri == len(runs) - 1
        b = 0
        while b < nb:
            rem = nb - b
            if last_run and rem <= MG_BLOCKS:
                if rem > 2:
                    g = rem - 2
                else:
                    g = 1
            else:
                g = min(MG_BLOCKS, rem)
            groups.append((ri, g))
            b += g
    return groups


def _build_nc(n_blocks_core, runs, n_slots):
    T_core = n_blocks_core * PB
    f32 = mybir.dt.float32
    bf16 = mybir.dt.bfloat16
    groups = _run_groups(runs)

    nc = bacc.Bacc("TRN2", target_bir_lowering=False, debug=False, num_devices=NCORES)
    xh = nc.dram_tensor("xh", [len(groups), PB, KT, MGT], bf16, kind="ExternalInput")
    w = nc.dram_tensor("w", [n_slots, NTILES, PB, KT, NT], bf16, kind="ExternalInput")
    out = nc.dram_tensor("out", [T_core, D_OUT], f32, kind="ExternalOutput")

    with TileContext(nc) as tc:
        with (
            tc.tile_pool(name="wpool", bufs=4) as wpool,
            tc.tile_pool(name="xpool", bufs=6) as xpool,
            tc.tile_pool(name="opool", bufs=8) as opool,
            tc.tile_pool(name="pspool", bufs=7, space="PSUM") as pspool,
            tc.tile_pool(name="warmpool", bufs=1, space="PSUM") as warmpool,
        ):
            def emit_w(slot, first_run):
                wt, w_dmas = [], []
                for n in range(NTILES):
                    t = wpool.tile(
                        [PB, KT, NT], bf16, name=f"w_s{slot}_n{n}", tag="w"
                    )
                    kh = KT // 2
                    dmas = [
                        nc.sync.dma_start(out=t[:, :kh, :], in_=w[slot, n, :, :kh, :]),
                        nc.gpsimd.dma_start(
                            out=t[:, kh:, :], in_=w[slot, n, :, kh:, :]
                        ),
                    ]
                    wt.append(t)
                    w_dmas.append(dmas)
                return wt, w_dmas

            wt0, w_dmas0 = emit_w(runs[0][0], True)

            wsrc = xpool.tile([PB, 2 * PB], bf16, name="warm_src", tag="warm")
            nc.vector.memset(wsrc, 0.0)
            wps = warmpool.tile([PB, 2 * PB], f32, name="warm_ps", tag="warm_ps")
            for _ in range(48):
                nc.tensor.matmul(wps, wsrc[:, :PB], wsrc, start=True, stop=True)

            ganchors = []
            run_group0 = []
            g0 = 0
            for ri in range(len(runs)):
                run_group0.append(g0)
                g0 += sum(1 for rr, _ in groups if rr == ri)

            n_groups_total = len(groups)
            blk = 0
            for ri, (slot, nb) in enumerate(runs):
                if ri == 0:
                    wt, w_dmas = wt0, w_dmas0
                else:
                    wt, w_dmas = emit_w(slot, False)
                pass_sets = [[0, 1], [2, 3]]
                chunk_first_mm = {}
                deferred = []
                for p, nset in enumerate(pass_sets):
                    gi = run_group0[ri]
                    chunk = 0
                    b = 0
                    while b < nb:
                        _, g = groups[gi]
                        xt = xpool.tile(
                            [PB, KT, MGT], bf16, name=f"xt_{gi}_{p}", tag="xt"
                        )
                        gchunk = len(ganchors)
                        if gchunk == 0:
                            kh = KT // 2
                            x_dmas = [
                                nc.scalar.dma_start(
                                    out=xt[:, :kh, :PB], in_=xh[gi][:, :kh, :PB]
                                ),
                                nc.scalar.dma_start(
                                    out=xt[:, kh:, :PB], in_=xh[gi][:, kh:, :PB]
                                ),
                                nc.scalar.dma_start(
                                    out=xt[:, :, PB:], in_=xh[gi][:, :, PB:]
                                ),
                            ]
                        else:
                            x_dmas = [nc.scalar.dma_start(out=xt, in_=xh[gi])]
                        if gchunk >= 2:
                            for dd in x_dmas:
                                add_dep_helper(
                                    dd.ins,
                                    ganchors[gchunk - 2].ins,
                                    sync=True,
                                    reason="pace x prefetch behind compute",
                                )
                        ots = []
                        for mb in range(g):
                            ot = opool.tile(
                                [PB, len(nset) * NT],
                                f32,
                                name=f"o_{blk + b + mb}_{p}",
                                tag="o",
                            )
                            ots.append(ot)

                        def emit_groups(
                            xt_, ots_, g_, js, base_b, first_anchor, split_out=False
                        ):
                            first_mm = None
                            for j in js:
                                n = nset[j]
                                for mb in range(g_):
                                    ps = pspool.tile(
                                        [PB, NT], f32, name="ps", tag="ps"
                                    )
                                    for k in range(KT):
                                        mm = nc.tensor.matmul(
                                            ps,
                                            xt_[:, k, mb * PB : (mb + 1) * PB],
                                            wt[n][:, k, :],
                                            start=(k == 0),
                                            stop=(k == KT - 1),
                                        )
                                        if first_mm is None:
                                            first_mm = mm
                                        if first_anchor and j == js[0] and mb == 0 and k == 0:
                                            chunk_first_mm[first_anchor[0]] = mm
                                    if split_out:
                                        row = (blk + base_b + mb) * PB
                                        hn = NT // 2
                                        for h in range(2):
                                            c0 = j * NT + h * hn
                                            o0 = n * NT + h * hn
                                            nc.vector.tensor_copy(
                                                out=ots_[mb][:, c0 : c0 + hn],
                                                in_=ps[:, h * hn : (h + 1) * hn],
                                            )
                                            nc.scalar.dma_start(
                                                out=out[row : row + PB, o0 : o0 + hn],
                                                in_=ots_[mb][:, c0 : c0 + hn],
                                            )
                                    else:
                                        nc.vector.tensor_copy(
                                            out=ots_[mb][:, j * NT : (j + 1) * NT],
                                            in_=ps,
                                        )
                            return first_mm

                        def emit_outs(ots_, g_, base_b):
                            for mb in range(g_):
                                row = (blk + base_b + mb) * PB
                                nc.scalar.dma_start(
                                    out=out[
                                        row : row + PB,
                                        nset[0] * NT : (nset[-1] + 1) * NT,
                                    ],
                                    in_=ots_[mb],
                                )

                        all_js = list(range(len(nset)))
                        tail_split = (
                            ri == len(runs) - 1
                            and p == len(pass_sets) - 1
                            and gi >= n_groups_total - 2
                        )
                        if (
                            ri == 0
                            and p == 0
                            and chunk < DEFER
                            and len(nset) > 1
                            and all(
                                run_group0[0] + c < len(groups)
                                and groups[run_group0[0] + c][0] == 0
                                for c in range(DEFER + 1)
                            )
                        ):
                            fm = emit_groups(xt, ots, g, all_js[:1], b, (chunk,))
                            ganchors.append(fm)
                            deferred.append((xt, ots, g, b))
                        else:
                            fm = emit_groups(
                                xt,
                                ots,
                                g,
                                all_js,
                                b,
                                (chunk,) if p == 0 else None,
                                split_out=tail_split,
                            )
                            ganchors.append(fm)
                            if not tail_split:
                                emit_outs(ots, g, b)
                            for xt_, ots_, g_, b_ in deferred:
                                emit_groups(xt_, ots_, g_, all_js[1:], b_, None)
                                emit_outs(ots_, g_, b_)
                            deferred = []
                        gi += 1
                        chunk += 1
                        b += g
                nchunks = chunk
                staggered = ((1, 0), (2, 1), (3, 2)) if ri == 0 else ((2, 1), (3, 2))
                for n, anchor in staggered:
                    a = chunk_first_mm.get(min(anchor, nchunks - 1))
                    if a is not None and nchunks > 2:
                        for dd in w_dmas[n]:
                            add_dep_helper(
                                dd.ins,
                                a.ins,
                                sync=True,
                                reason="stagger weight n-set stream behind ramp",
                            )
                blk += nb
    nc.compile()
    return nc


def _host_layout_x(x_core, runs):
    groups = _run_groups(runs)
    x16 = np.asarray(x_core, dtype=BF16)
    xh = np.zeros((len(groups), PB, KT, MGT), dtype=BF16)
    t0 = 0
    for i, (_, g) in enumerate(groups):
        gt = g * PB
        blockT = x16[t0 : t0 + gt]
        xh[i, :, :, :gt] = blockT.reshape(gt, KT, PB).transpose(2, 1, 0)
        t0 += gt
    return np.ascontiguousarray(xh)


def _host_layout_w(w_slots):
    S = w_slots.shape[0]
    w16 = np.asarray(w_slots, dtype=BF16)
    return np.ascontiguousarray(
        w16.reshape(S, KT, PB, NTILES, NT).transpose(0, 3, 2, 1, 4)
    )


def _kernel_general(x, weight, seg):
    global LAST_RESULTS
    aligned = all(
        np.all(seg[i * PB : (i + 1) * PB] == seg[i * PB]) for i in range(N_TOK // PB)
    )
    if aligned:
        block_expert = seg[::PB].astype(np.int64)
        block_tokens = None
    else:
        bounds = np.flatnonzero(np.diff(seg)) + 1
        starts = np.concatenate([[0], bounds])
        ends = np.concatenate([bounds, [N_TOK]])
        blocks, experts = [], []
        for s, e in zip(starts, ends):
            idx = np.arange(s, e, dtype=np.int64)
            padded = -np.ones(int(np.ceil(len(idx) / PB)) * PB, dtype=np.int64)
            padded[: len(idx)] = idx
            for b0 in range(0, len(padded), PB):
                blocks.append(padded[b0 : b0 + PB])
                experts.append(int(seg[s]))
        while len(blocks) % NCORES:
            blocks.append(-np.ones(PB, dtype=np.int64))
            experts.append(0)
        block_tokens = np.stack(blocks)
        block_expert = np.asarray(experts, dtype=np.int64)

    n_blocks = len(block_expert)
    n_blocks_core = n_blocks // NCORES
    per_core_experts = block_expert.reshape(NCORES, n_blocks_core)

    def rle(v):
        runs = []
        for e in v:
            if runs and runs[-1][0] == e:
                runs[-1][1] += 1
            else:
                runs.append([int(e), 1])
        return runs

    core_runs = [rle(per_core_experts[c]) for c in range(NCORES)]
    lengths0 = [n for _, n in core_runs[0]]
    if all([n for _, n in core_runs[c]] == lengths0 for c in range(NCORES)):
        runs = [(s, n) for s, (_, n) in enumerate(core_runs[0])]
        slot_experts = [[e for e, _ in core_runs[c]] for c in range(NCORES)]
    else:
        runs = [(b, 1) for b in range(n_blocks_core)]
        slot_experts = [list(per_core_experts[c]) for c in range(NCORES)]
    n_slots = len(runs)

    in_maps = []
    for c in range(NCORES):
        if block_tokens is None:
            rows = slice(c * n_blocks_core * PB, (c + 1) * n_blocks_core * PB)
            xc = x[rows]
        else:
            tok = block_tokens[c * n_blocks_core : (c + 1) * n_blocks_core].ravel()
            xc = np.where(tok[:, None] >= 0, x[np.maximum(tok, 0)], 0.0).astype(
                np.float32
            )
        in_maps.append(
            {
                "xh": _host_layout_x(xc, runs),
                "w": _host_layout_w(weight[slot_experts[c]]),
            }
        )

    nc = _build_nc(n_blocks_core, runs, n_slots)
    res = run_bass_kernel_spmd(nc, in_maps, core_ids=list(range(NCORES)), trace=TRACE)
    LAST_RESULTS = res

    outs = [res.results[c]["out"] for c in range(NCORES)]
    if block_tokens is None:
        return np.concatenate(outs, axis=0)
    full = np.zeros((N_TOK, D_OUT), dtype=np.float32)
    flat_tok = block_tokens.ravel()
    flat_out = np.concatenate(outs, axis=0)
    valid = flat_tok >= 0
    full[flat_tok[valid]] = flat_out[valid]
    return full


def kernel(x, weight, num_inputs_per_group):
    x = np.ascontiguousarray(np.asarray(x, dtype=np.float32))
    weight = np.ascontiguousarray(np.asarray(weight, dtype=np.float32))
    num = np.asarray(num_inputs_per_group)
    seg = _seg_from_groups(num)
    if np.all(np.asarray(num, dtype=np.int64) == N_TOK // N_EXP):
        return _kernel_strassen(x, weight)
    return _kernel_general(x, weight, seg)


# revision 20
# speedup vs baseline: 1.0701x; 1.0701x over previous
"""Trainium2 Bass kernel: batch row-sharded grouped GEMM (MoE routing).

Contract: kernel(x, weight, num_inputs_per_group) takes FULL inputs
  x (32768, 2048) f32, weight (16, 2048, 2048) f32, num_inputs_per_group (16,) i32
and returns the FULL output (32768, 2048) f32, where token row i is multiplied
by weight[seg[i]] with seg = repeat(arange(16), num, total_repeat_length=32768)
(contiguous groups).

Distribution strategy (no collectives needed): tokens are split contiguously;
each of the 8 cores gets 4096 tokens plus the weights of the experts those
tokens use (expert/token parallelism — sanctioned by the sharding hint since
E=16 >= 8). Each core computes its grouped GEMM locally and the host
concatenates the per-core outputs.

Fast path (uniform 2048-token groups, the shipped distribution): one level of
STRASSEN per (2048 x 2048) @ (2048 x 2048) expert GEMM. Each run (one expert,
2048 tokens) is split into 2x2 blocks of 1024: 7 products instead of 8 cuts
the PE matmul floor by 12.5% (442us -> 387us across 8 cores); the extra
elementwise adds ride on the otherwise-idle Vector/Scalar/GpSimd engines,
fully hidden under PE compute.

Device schedule per run — multiply-index OUTER (stage), token-block-pair
inner, so each 2 MiB weight tensor is consumed over a full 27.6us stage
(76 GB/s demand; no HBM ramp spike) and dies at stage end (wpool bufs=3):
  stage: 0:M4=A22*T4  1:M3=A11*T3  2:M5=S3*B22  3:M2=S2*B11
         4:M1=S1*T1   5:M7=S5*T7   6:M6=S4*T6
  (T1=B11+B22, T3=B12-B22, T4=B21-B11, T6=B11+B12, T7=B21+B22 precomputed
   on HOST in fp32, shipped bf16 — 7 weight tensors of [1024,1024] per run.)
Pairing: token block p (rows p*128 of the run's first 1024) pairs with block
p+8 (second 1024); S-prep sums (GpSimd, bf16) and partial combines (Vector,
reading PSUM fp32 directly, bf16 partials) follow Strassen:
  s4=cp(M4); m3cp=cp(M3); C12=m3cp+M5, c11p=s4-M5; C21=s4+M2, c22b=m3cp-M2;
  t1=c11p+M1, t22=c22b+M1; C11=t1+M7; C22=t22+M6.
Copies on Scalar (ACT), preps on GpSimd, combines on Vector — each <=80% busy
per stage. Outputs drain as bf16 (host upcasts); rel err ~5.9e-3 (gate 2e-2).
Rings: sync = weights (28 MiB), scalar = x in + out (24 MiB) — no saturated
queue (baseline's scalar ring carried 65 MiB at ~150 GB/s limit).
Ramp: first stage needs only x's A22 quarter (0.125 MiB, first scalar job)
plus T4's first k-quarter (0.5 MiB, first sync job); ~14 throwaway bf16
warmup matmuls bridge the ~8.2us SPMD preamble into first real work while
holding the PE HAM clock at 8/8.
"""

import sys

sys.path.insert(0, "/opt/trn_rl_repo")

import numpy as np
import ml_dtypes

import concourse.bacc as bacc
import concourse.mybir as mybir
from concourse.bass_utils import run_bass_kernel_spmd
from concourse.tile import TileContext
from concourse.tile_rust import add_dep_helper

BF16 = ml_dtypes.bfloat16

N_TOK, D_IN, D_OUT, N_EXP = 32768, 2048, 2048, 16
NCORES = 8
PB = 128  # token block = PSUM partition count

# ---- Strassen fast-path geometry ----
RUNS = 2          # experts (runs) per core
RUN_T = 2048      # tokens per run
NPAIR = 8         # 128-token block pairs per run (block p with block p+8)
KO = 8            # k-outer tiles per 1024-half (8 x 128)
NH = 512          # PSUM tile width (one bank)
N_WARM = 12

# Introspection hooks for test.py (harness just calls kernel()).
TRACE = False
LAST_RESULTS = None


def _seg_from_groups(num):
    """Replicate jnp.repeat(arange(E), num, total_repeat_length=N) semantics."""
    num = np.asarray(num, dtype=np.int64)
    reps = np.repeat(np.arange(N_EXP, dtype=np.int32), np.maximum(num, 0))
    if len(reps) >= N_TOK:
        return reps[:N_TOK]
    pad = reps[-1] if len(reps) else np.int32(0)
    return np.concatenate([reps, np.full(N_TOK - len(reps), pad, np.int32)])


# ===========================================================================
# Strassen fast path (uniform 2048-token groups -> 2 experts per core)
# ===========================================================================

# stage -> (host weight matrix index, stationary kind)
#   stationary kinds: 'A22', 'A11' read x tile slices directly;
#   'S3','S2','S1','S5','S4' are GpSimd-prepped sums.
_STAGES = [
    ("A22", None),  # M4 = A22 @ T4
    ("A11", None),  # M3 = A11 @ T3
    ("S3", "add_00_10"),   # M5 = (A11+A12) @ B22
    ("S2", "add_01_11"),   # M2 = (A21+A22) @ B11
    ("S1", "add_00_11"),   # M1 = (A11+A22) @ T1
    ("S5", "sub_10_11"),   # M7 = (A12-A22) @ T7
    ("S4", "sub_01_00"),   # M6 = (A21-A11) @ T6
]
# x tile layout [128(d%128), 16(d//128), 256(tok: r1 128 | r2 128)]:
#   A11 = [:, 0:8, 0:128]    A12 = [:, 8:16, 0:128]
#   A21 = [:, 0:8, 128:256]  A22 = [:, 8:16, 128:256]
_XSLICE = {
    "A11": (0, 0), "A12": (8, 0), "A21": (0, 128), "A22": (8, 128),
}
_PREP = {  # S kind -> (lhs slice, rhs slice, is_sub)
    "S3": ("A11", "A12", False),
    "S2": ("A21", "A22", False),
    "S1": ("A11", "A22", False),
    "S5": ("A12", "A22", True),
    "S4": ("A21", "A11", True),
}


def _build_nc_strassen():
    f32 = mybir.dt.float32
    bf16 = mybir.dt.bfloat16

    nc = bacc.Bacc("TRN2", target_bir_lowering=False, debug=False, num_devices=NCORES)
    xh = nc.dram_tensor("xh", [RUNS, NPAIR, PB, 2 * KO, 2 * PB], bf16,
                        kind="ExternalInput")
    wh = nc.dram_tensor("wh", [RUNS, 7, PB, KO, 2 * NH], bf16,
                        kind="ExternalInput")
    out = nc.dram_tensor("out", [RUNS * RUN_T, D_OUT], bf16, kind="ExternalOutput")

    with TileContext(nc) as tc:
        with (
            tc.tile_pool(name="wpool", bufs=4) as wpool,
            tc.tile_pool(name="xpool", bufs=8) as xpool,
            tc.tile_pool(name="spool", bufs=5) as spool,
            tc.tile_pool(name="ppool", bufs=26) as ppool,
            tc.tile_pool(name="opool", bufs=6) as opool,
            tc.tile_pool(name="pspool", bufs=8, space="PSUM") as pspool,
        ):
            # ---- weight streams: all 14 tensors upfront on the sync ring.
            # wpool bufs=3 + engine FIFO = natural just-in-time pacing with
            # ~2 stages of lookahead. k-quarter split gives the ramp
            # fine-grained deps. The very first tensor (run0 T4) is split
            # k-eighth on sync plus k-quarters on the otherwise-idle gpsimd
            # ring so the ramp's first bytes land at ~2x bandwidth (these
            # have no pool-reuse waits, so they cannot block either FIFO).
            wtiles = [[None] * 7 for _ in range(RUNS)]
            for r in range(RUNS):
                for si in range(7):
                    t = wpool.tile([PB, KO, 2 * NH], bf16, name=f"w_{r}_{si}",
                                   tag="w")
                    if r == 0 and si == 0:
                        # First tensor feeds the ramp: the consumption order
                        # is (nh0: k0..7), (nh1: k0..7), so deliver nh0 via
                        # two queues in parallel, then nh1 the same way.
                        nc.sync.dma_start(out=t[:, 0:2, :NH], in_=wh[r, si][:, 0:2, :NH])
                        nc.sync.dma_start(out=t[:, 2:4, :NH], in_=wh[r, si][:, 2:4, :NH])
                        nc.gpsimd.dma_start(out=t[:, 4:6, :NH], in_=wh[r, si][:, 4:6, :NH])
                        nc.gpsimd.dma_start(out=t[:, 6:8, :NH], in_=wh[r, si][:, 6:8, :NH])
                        nc.sync.dma_start(out=t[:, 0:4, NH:], in_=wh[r, si][:, 0:4, NH:])
                        nc.gpsimd.dma_start(out=t[:, 4:8, NH:], in_=wh[r, si][:, 4:8, NH:])
                    elif r == 0 and si == 1:
                        # second tensor also rides both ramp queues so it
                        # lands before stage 1 starts (~27us in)
                        nc.sync.dma_start(out=t[:, 0:2, :], in_=wh[r, si][:, 0:2, :])
                        nc.sync.dma_start(out=t[:, 2:4, :], in_=wh[r, si][:, 2:4, :])
                        nc.gpsimd.dma_start(out=t[:, 4:6, :], in_=wh[r, si][:, 4:6, :])
                        nc.gpsimd.dma_start(out=t[:, 6:8, :], in_=wh[r, si][:, 6:8, :])
                    else:
                        for q in range(4):
                            nc.sync.dma_start(
                                out=t[:, 2 * q:2 * q + 2, :],
                                in_=wh[r, si][:, 2 * q:2 * q + 2, :],
                            )
                    wtiles[r][si] = t

            # ---- x streams: run 0 entirely on the scalar ring (A22 quarters
            # of every pair first — stage 0's only need — then the rest).
            # Run r+1's A22 quarters are issued from the GPSIMD ring inside
            # run r's stage-6 prep loop, each right after the S4 prep that
            # is the old tile's last reader — so the xpool-reuse wait is
            # satisfied by the immediately preceding same-engine instruction
            # and never blocks a FIFO. Run r+1's remaining x goes on scalar
            # at its run head (waits already satisfied by then).
            xtiles = [[None] * NPAIR for _ in range(RUNS)]
            for p in range(NPAIR):
                t = xpool.tile([PB, 2 * KO, 2 * PB], bf16,
                               name=f"x_0_{p}", tag="x")
                nc.scalar.dma_start(out=t[:, KO:, PB:], in_=xh[0, p][:, KO:, PB:])
                xtiles[0][p] = t
            for p in range(NPAIR):
                t = xtiles[0][p]
                nc.scalar.dma_start(out=t[:, :KO, :], in_=xh[0, p][:, :KO, :])
                nc.scalar.dma_start(out=t[:, KO:, :PB], in_=xh[0, p][:, KO:, :PB])

            # ---- PE warmup: throwaway bf16 matmuls bridge the SPMD preamble
            # until the first weight quarter lands, holding HAM at 8/8.
            wsrc = spool.tile([PB, 2 * PB], bf16, name="warm_src", tag="warm")
            nc.vector.memset(wsrc, 0.0)
            wps = pspool.tile([PB, NH], f32, name="warm_ps", tag="ps")
            for _ in range(N_WARM):
                nc.tensor.matmul(wps[:, :2 * PB], wsrc[:, :PB], wsrc,
                                 start=True, stop=True)

            def xsl(xt, kind, k=None):
                ko0, t0 = _XSLICE[kind]
                if k is None:
                    return xt[:, ko0:ko0 + KO, t0:t0 + PB]
                return xt[:, ko0 + k, t0:t0 + PB]

            for r in range(RUNS):
                rb = r * RUN_T
                if r > 0:
                    for p in range(NPAIR):
                        t = xtiles[r][p]
                        nc.scalar.dma_start(out=t[:, :KO, :], in_=xh[r, p][:, :KO, :])
                        nc.scalar.dma_start(out=t[:, KO:, :PB],
                                            in_=xh[r, p][:, KO:, :PB])
                # per-pair partials (bf16 [128, 1024]) living across stages
                s4 = [None] * NPAIR
                m3cp = [None] * NPAIR
                c11p = [None] * NPAIR
                c22b = [None] * NPAIR
                t1 = [None] * NPAIR
                t22 = [None] * NPAIR
                for si, (stat, prep) in enumerate(_STAGES):
                    wt = wtiles[r][si]
                    # S-preps for this stage (GpSimd, bf16): emitted ahead of
                    # the consuming pair's matmuls; gpsimd runs ~1 pair ahead.
                    stiles = [None] * NPAIR
                    if prep is not None:
                        a, b, is_sub = _PREP[stat]
                        for p in range(NPAIR):
                            st = spool.tile([PB, KO, PB], bf16,
                                            name=f"s_{r}_{si}_{p}", tag="s")
                            xt = xtiles[r][p]
                            if is_sub:
                                nc.gpsimd.tensor_sub(st, xsl(xt, a), xsl(xt, b))
                            else:
                                nc.gpsimd.tensor_add(st, xsl(xt, a), xsl(xt, b))
                            stiles[p] = st
                            if si == 6 and r + 1 < RUNS:
                                # next run's stage-0 x quarter, right behind
                                # this tile's final reader (the prep above)
                                tn = xpool.tile([PB, 2 * KO, 2 * PB], bf16,
                                                name=f"x_{r + 1}_{p}", tag="x")
                                nc.gpsimd.dma_start(
                                    out=tn[:, KO:, PB:],
                                    in_=xh[r + 1, p][:, KO:, PB:])
                                xtiles[r + 1][p] = tn
                    for p in range(NPAIR):
                        xt = xtiles[r][p]
                        ps = []
                        for nh in range(2):
                            pst = pspool.tile([PB, NH], f32, name="ps", tag="ps")
                            for k in range(KO):
                                if prep is None:
                                    lhsT = xsl(xt, stat, k)
                                else:
                                    lhsT = stiles[p][:, k, :]
                                nc.tensor.matmul(
                                    pst, lhsT, wt[:, k, nh * NH:(nh + 1) * NH],
                                    start=(k == 0), stop=(k == KO - 1),
                                )
                            ps.append(pst)
                        r1 = rb + p * PB
                        r2 = rb + RUN_T // 2 + p * PB
                        if si == 0:  # M4 -> s4 = cp(M4)   (vector is idle here)
                            s4[p] = ppool.tile([PB, 2 * NH], bf16,
                                               name=f"s4_{r}_{p}", tag="pp")
                            for h in range(2):
                                nc.vector.tensor_copy(
                                    out=s4[p][:, h * NH:(h + 1) * NH], in_=ps[h])
                        elif si == 1:  # M3 -> m3cp = cp(M3)
                            m3cp[p] = ppool.tile([PB, 2 * NH], bf16,
                                                 name=f"m3_{r}_{p}", tag="pp")
                            for h in range(2):
                                nc.vector.tensor_copy(
                                    out=m3cp[p][:, h * NH:(h + 1) * NH], in_=ps[h])
                        elif si == 2:  # M5: C12 = m3cp + M5 ; c11p = s4 - M5
                            ot = opool.tile([PB, 2 * NH], bf16,
                                            name=f"c12_{r}_{p}", tag="o")
                            c11p[p] = ppool.tile([PB, 2 * NH], bf16,
                                                 name=f"c11p_{r}_{p}", tag="pp")
                            for h in range(2):
                                hs = slice(h * NH, (h + 1) * NH)
                                nc.vector.tensor_add(ot[:, hs], m3cp[p][:, hs], ps[h])
                                nc.vector.tensor_sub(c11p[p][:, hs], s4[p][:, hs], ps[h])
                            nc.scalar.dma_start(
                                out=out[r1:r1 + PB, D_OUT // 2:], in_=ot)
                        elif si == 3:  # M2: C21 = s4 + M2 ; c22b = m3cp - M2
                            ot = opool.tile([PB, 2 * NH], bf16,
                                            name=f"c21_{r}_{p}", tag="o")
                            c22b[p] = ppool.tile([PB, 2 * NH], bf16,
                                                 name=f"c22b_{r}_{p}", tag="pp")
                            for h in range(2):
                                hs = slice(h * NH, (h + 1) * NH)
                                nc.vector.tensor_add(ot[:, hs], s4[p][:, hs], ps[h])
                                nc.vector.tensor_sub(c22b[p][:, hs], m3cp[p][:, hs], ps[h])
                            nc.scalar.dma_start(
                                out=out[r2:r2 + PB, :D_OUT // 2], in_=ot)
                        elif si == 4:  # M1: t1 = c11p + M1 ; t22 = c22b + M1
                            t1[p] = ppool.tile([PB, 2 * NH], bf16,
                                               name=f"t1_{r}_{p}", tag="pp")
                            t22[p] = ppool.tile([PB, 2 * NH], bf16,
                                                name=f"t22_{r}_{p}", tag="pp")
                            for h in range(2):
                                hs = slice(h * NH, (h + 1) * NH)
                                nc.vector.tensor_add(t1[p][:, hs], c11p[p][:, hs], ps[h])
                                nc.vector.tensor_add(t22[p][:, hs], c22b[p][:, hs], ps[h])
                        elif si == 5:  # M7: C11 = t1 + M7
                            ot = opool.tile([PB, 2 * NH], bf16,
                                            name=f"c11_{r}_{p}", tag="o")
                            for h in range(2):
                                hs = slice(h * NH, (h + 1) * NH)
                                nc.vector.tensor_add(ot[:, hs], t1[p][:, hs], ps[h])
                            nc.scalar.dma_start(
                                out=out[r1:r1 + PB, :D_OUT // 2], in_=ot)
                        else:  # si == 6, M6: C22 = t22 + M6
                            ot = opool.tile([PB, 2 * NH], bf16,
                                            name=f"c22_{r}_{p}", tag="o")
                            last = r == RUNS - 1 and p == NPAIR - 1
                            for h in range(2):
                                hs = slice(h * NH, (h + 1) * NH)
                                nc.vector.tensor_add(ot[:, hs], t22[p][:, hs], ps[h])
                                if last:
                                    # drain the kernel's final tile per half so
                                    # the last DMA overlaps the last vec op
                                    nc.scalar.dma_start(
                                        out=out[r2:r2 + PB,
                                                D_OUT // 2 + h * NH:
                                                D_OUT // 2 + (h + 1) * NH],
                                        in_=ot[:, hs])
                            if not last:
                                nc.scalar.dma_start(
                                    out=out[r2:r2 + PB, D_OUT // 2:], in_=ot)
    nc.compile()
    return nc


def _host_pack_x_strassen(xc):
    """Pack a core's 4096 tokens into bf16 pair tiles [2, 8, 128, 16, 256]."""
    x16 = np.asarray(xc, dtype=BF16)
    xhp = np.empty((RUNS, NPAIR, PB, 2 * KO, 2 * PB), dtype=BF16)
    for r in range(RUNS):
        xr = x16[r * RUN_T:(r + 1) * RUN_T]
        for p in range(NPAIR):
            r1 = xr[p * PB:(p + 1) * PB]
            r2 = xr[RUN_T // 2 + p * PB:RUN_T // 2 + (p + 1) * PB]
            blk = np.concatenate([r1, r2], axis=0)  # [256, 2048]
            # (tok, d) -> (d%128, d//128, tok)
            xhp[r, p] = blk.reshape(2 * PB, 2 * KO, PB).transpose(2, 1, 0)
    return np.ascontiguousarray(xhp)


def _host_pack_w_strassen(experts_w):
    """Pack per-run expert weights [2, 2048, 2048] f32 into the 7 Strassen
    weight tensors per run, bf16 tiles [2, 7, 128, 8, 1024] (stage order)."""
    whp = np.empty((RUNS, 7, PB, KO, 2 * NH), dtype=BF16)
    for r in range(RUNS):
        we = experts_w[r]
        h = D_IN // 2
        B11, B12 = we[:h, :h], we[:h, h:]
        B21, B22 = we[h:, :h], we[h:, h:]
        mats = [B21 - B11, B12 - B22, B22, B11, B11 + B22, B21 + B22, B11 + B12]
        for si, m in enumerate(mats):
            # (k, n) -> (k%128, k//128, n)
            whp[r, si] = m.astype(BF16).reshape(KO, PB, 2 * NH).transpose(1, 0, 2)
    return np.ascontiguousarray(whp)


def _kernel_strassen(x, weight):
    global LAST_RESULTS
    in_maps = []
    for c in range(NCORES):
        xc = x[c * RUNS * RUN_T:(c + 1) * RUNS * RUN_T]
        ew = weight[[2 * c, 2 * c + 1]]
        in_maps.append({
            "xh": _host_pack_x_strassen(xc),
            "wh": _host_pack_w_strassen(ew),
        })
    nc = _build_nc_strassen()
    res = run_bass_kernel_spmd(nc, in_maps, core_ids=list(range(NCORES)),
                               trace=TRACE)
    LAST_RESULTS = res
    outs = [np.asarray(res.results[c]["out"], dtype=np.float32)
            for c in range(NCORES)]
    return np.concatenate(outs, axis=0)


# ===========================================================================
# General fallback path (previous-generation kernel, handles any grouping)
# ===========================================================================

NT = 512  # matmul moving free dim = one fp32 PSUM bank
KT = D_IN // PB  # 16 k-tiles
NTILES = D_OUT // NT  # 4 output column sets
MG_BLOCKS = 2  # token blocks per x group tile
MGT = MG_BLOCKS * PB  # tokens per group tile
DEFER = 2  # chunks of run 0 whose n1 work is deferred past chunk DEFER's


def _run_groups(runs):
    groups = []  # (run_idx, g_blocks)
    for ri, (_, nb) in enumerate(runs):
        last_run = ri == len(runs) - 1
        b = 0
        while b < nb:
            rem = nb - b
            if last_run and rem <= MG_BLOCKS:
                if rem > 2:
                    g = rem - 2
                else:
                    g = 1
            else:
                g = min(MG_BLOCKS, rem)
            groups.append((ri, g))
            b += g
    return groups


def _build_nc(n_blocks_core, runs, n_slots):
    T_core = n_blocks_core * PB
    f32 = mybir.dt.float32
    bf16 = mybir.dt.bfloat16
    groups = _run_groups(runs)

    nc = bacc.Bacc("TRN2", target_bir_lowering=False, debug=False, num_devices=NCORES)
    xh = nc.dram_tensor("xh", [len(groups), PB, KT, MGT], bf16, kind="ExternalInput")
    w = nc.dram_tensor("w", [n_slots, NTILES, PB, KT, NT], bf16, kind="ExternalInput")
    out = nc.dram_tensor("out", [T_core, D_OUT], f32, kind="ExternalOutput")

    with TileContext(nc) as tc:
        with (
            tc.tile_pool(name="wpool", bufs=4) as wpool,
            tc.tile_pool(name="xpool", bufs=6) as xpool,
            tc.tile_pool(name="opool", bufs=8) as opool,
            tc.tile_pool(name="pspool", bufs=7, space="PSUM") as pspool,
            tc.tile_pool(name="warmpool", bufs=1, space="PSUM") as warmpool,
        ):
            def emit_w(slot, first_run):
                wt, w_dmas = [], []
                for n in range(NTILES):
                    t = wpool.tile(
                        [PB, KT, NT], bf16, name=f"w_s{slot}_n{n}", tag="w"
                    )
                    kh = KT // 2
                    dmas = [
                        nc.sync.dma_start(out=t[:, :kh, :], in_=w[slot, n, :, :kh, :]),
                        nc.gpsimd.dma_start(
                            out=t[:, kh:, :], in_=w[slot, n, :, kh:, :]
                        ),
                    ]
                    wt.append(t)
                    w_dmas.append(dmas)
                return wt, w_dmas

            wt0, w_dmas0 = emit_w(runs[0][0], True)

            wsrc = xpool.tile([PB, 2 * PB], bf16, name="warm_src", tag="warm")
            nc.vector.memset(wsrc, 0.0)
            wps = warmpool.tile([PB, 2 * PB], f32, name="warm_ps", tag="warm_ps")
            for _ in range(48):
                nc.tensor.matmul(wps, wsrc[:, :PB], wsrc, start=True, stop=True)

            ganchors = []
            run_group0 = []
            g0 = 0
            for ri in range(len(runs)):
                run_group0.append(g0)
                g0 += sum(1 for rr, _ in groups if rr == ri)

            n_groups_total = len(groups)
            blk = 0
            for ri, (slot, nb) in enumerate(runs):
                if ri == 0:
                    wt, w_dmas = wt0, w_dmas0
                else:
                    wt, w_dmas = emit_w(slot, False)
                pass_sets = [[0, 1], [2, 3]]
                chunk_first_mm = {}
                deferred = []
                for p, nset in enumerate(pass_sets):
                    gi = run_group0[ri]
                    chunk = 0
                    b = 0
                    while b < nb:
                        _, g = groups[gi]
                        xt = xpool.tile(
                            [PB, KT, MGT], bf16, name=f"xt_{gi}_{p}", tag="xt"
                        )
                        gchunk = len(ganchors)
                        if gchunk == 0:
                            kh = KT // 2
                            x_dmas = [
                                nc.scalar.dma_start(
                                    out=xt[:, :kh, :PB], in_=xh[gi][:, :kh, :PB]
                                ),
                                nc.scalar.dma_start(
                                    out=xt[:, kh:, :PB], in_=xh[gi][:, kh:, :PB]
                                ),
                                nc.scalar.dma_start(
                                    out=xt[:, :, PB:], in_=xh[gi][:, :, PB:]
                                ),
                            ]
                        else:
                            x_dmas = [nc.scalar.dma_start(out=xt, in_=xh[gi])]
                        if gchunk >= 2:
                            for dd in x_dmas:
                                add_dep_helper(
                                    dd.ins,
                                    ganchors[gchunk - 2].ins,
                                    sync=True,
                                    reason="pace x prefetch behind compute",
                                )
                        ots = []
                        for mb in range(g):
                            ot = opool.tile(
                                [PB, len(nset) * NT],
                                f32,
                                name=f"o_{blk + b + mb}_{p}",
                                tag="o",
                            )
                            ots.append(ot)

                        def emit_groups(
                            xt_, ots_, g_, js, base_b, first_anchor, split_out=False
                        ):
                            first_mm = None
                            for j in js:
                                n = nset[j]
                                for mb in range(g_):
                                    ps = pspool.tile(
                                        [PB, NT], f32, name="ps", tag="ps"
                                    )
                                    for k in range(KT):
                                        mm = nc.tensor.matmul(
                                            ps,
                                            xt_[:, k, mb * PB : (mb + 1) * PB],
                                            wt[n][:, k, :],
                                            start=(k == 0),
                                            stop=(k == KT - 1),
                                        )
                                        if first_mm is None:
                                            first_mm = mm
                                        if first_anchor and j == js[0] and mb == 0 and k == 0:
                                            chunk_first_mm[first_anchor[0]] = mm
                                    if split_out:
                                        row = (blk + base_b + mb) * PB
                                        hn = NT // 2
                                        for h in range(2):
                                            c0 = j * NT + h * hn
                                            o0 = n * NT + h * hn
                                            nc.vector.tensor_copy(
                                                out=ots_[mb][:, c0 : c0 + hn],
                                                in_=ps[:, h * hn : (h + 1) * hn],
                                            )
                                            nc.scalar.dma_start(
                                                out=out[row : row + PB, o0 : o0 + hn],
                                                in_=ots_[mb][:, c0 : c0 + hn],
                                            )
                                    else:
                                        nc.vector.tensor_copy(
                                            out=ots_[mb][:, j * NT : (j + 1) * NT],
                                            in_=ps,
                                        )
                            return first_mm

                        def emit_outs(ots_, g_, base_b):
                            for mb in range(g_):
                                row = (blk + base_b + mb) * PB
                                nc.scalar.dma_start(
                                    out=out[
                                        row : row + PB,
                                        nset[0] * NT : (nset[-1] + 1) * NT,
                                    ],
                                    in_=ots_[mb],
                                )

                        all_js = list(range(len(nset)))
                        tail_split = (
                            ri == len(runs) - 1
                            and p == len(pass_sets) - 1
                            and gi >= n_groups_total - 2
                        )
                        if (
                            ri == 0
                            and p == 0
                            and chunk < DEFER
                            and len(nset) > 1
                            and all(
                                run_group0[0] + c < len(groups)
                                and groups[run_group0[0] + c][0] == 0
                                for c in range(DEFER + 1)
                            )
                        ):
                            fm = emit_groups(xt, ots, g, all_js[:1], b, (chunk,))
                            ganchors.append(fm)
                            deferred.append((xt, ots, g, b))
                        else:
                            fm = emit_groups(
                                xt,
                                ots,
                                g,
                                all_js,
                                b,
                                (chunk,) if p == 0 else None,
                                split_out=tail_split,
                            )
                            ganchors.append(fm)
                            if not tail_split:
                                emit_outs(ots, g, b)
                            for xt_, ots_, g_, b_ in deferred:
                                emit_groups(xt_, ots_, g_, all_js[1:], b_, None)
                                emit_outs(ots_, g_, b_)
                            deferred = []
                        gi += 1
                        chunk += 1
                        b += g
                nchunks = chunk
                staggered = ((1, 0), (2, 1), (3, 2)) if ri == 0 else ((2, 1), (3, 2))
                for n, anchor in staggered:
                    a = chunk_first_mm.get(min(anchor, nchunks - 1))
                    if a is not None and nchunks > 2:
                        for dd in w_dmas[n]:
                            add_dep_helper(
                                dd.ins,
                                a.ins,
                                sync=True,
                                reason="stagger weight n-set stream behind ramp",
                            )
                blk += nb
    nc.compile()
    return nc


def _host_layout_x(x_core, runs):
    groups = _run_groups(runs)
    x16 = np.asarray(x_core, dtype=BF16)
    xh = np.zeros((len(groups), PB, KT, MGT), dtype=BF16)
    t0 = 0
    for i, (_, g) in enumerate(groups):
        gt = g * PB
        blockT = x16[t0 : t0 + gt]
        xh[i, :, :, :gt] = blockT.reshape(gt, KT, PB).transpose(2, 1, 0)
        t0 += gt
    return np.ascontiguousarray(xh)


def _host_layout_w(w_slots):
    S = w_slots.shape[0]
    w16 = np.asarray(w_slots, dtype=BF16)
    return np.ascontiguousarray(
        w16.reshape(S, KT, PB, NTILES, NT).transpose(0, 3, 2, 1, 4)
    )


def _kernel_general(x, weight, seg):
    global LAST_RESULTS
    aligned = all(
        np.all(seg[i * PB : (i + 1) * PB] == seg[i * PB]) for i in range(N_TOK // PB)
    )
    if aligned:
        block_expert = seg[::PB].astype(np.int64)
        block_tokens = None
    else:
        bounds = np.flatnonzero(np.diff(seg)) + 1
        starts = np.concatenate([[0], bounds])
        ends = np.concatenate([bounds, [N_TOK]])
        blocks, experts = [], []
        for s, e in zip(starts, ends):
            idx = np.arange(s, e, dtype=np.int64)
            padded = -np.ones(int(np.ceil(len(idx) / PB)) * PB, dtype=np.int64)
            padded[: len(idx)] = idx
            for b0 in range(0, len(padded), PB):
                blocks.append(padded[b0 : b0 + PB])
                experts.append(int(seg[s]))
        while len(blocks) % NCORES:
            blocks.append(-np.ones(PB, dtype=np.int64))
            experts.append(0)
        block_tokens = np.stack(blocks)
        block_expert = np.asarray(experts, dtype=np.int64)

    n_blocks = len(block_expert)
    n_blocks_core = n_blocks // NCORES
    per_core_experts = block_expert.reshape(NCORES, n_blocks_core)

    def rle(v):
        runs = []
        for e in v:
            if runs and runs[-1][0] == e:
                runs[-1][1] += 1
            else:
                runs.append([int(e), 1])
        return runs

    core_runs = [rle(per_core_experts[c]) for c in range(NCORES)]
    lengths0 = [n for _, n in core_runs[0]]
    if all([n for _, n in core_runs[c]] == lengths0 for c in range(NCORES)):
        runs = [(s, n) for s, (_, n) in enumerate(core_runs[0])]
        slot_experts = [[e for e, _ in core_runs[c]] for c in range(NCORES)]
    else:
        runs = [(b, 1) for b in range(n_blocks_core)]
        slot_experts = [list(per_core_experts[c]) for c in range(NCORES)]
    n_slots = len(runs)

    in_maps = []
    for c in range(NCORES):
        if block_tokens is None:
            rows = slice(c * n_blocks_core * PB, (c + 1) * n_blocks_core * PB)
            xc = x[rows]
        else:
            tok = block_tokens[c * n_blocks_core : (c + 1) * n_blocks_core].ravel()
            xc = np.where(tok[:, None] >= 0, x[np.maximum(tok, 0)], 0.0).astype(
                np.float32
            )
        in_maps.append(
            {
                "xh": _host_layout_x(xc, runs),
                "w": _host_layout_w(weight[slot_experts[c]]),
            }
        )

    nc = _build_nc(n_blocks_core, runs, n_slots)
    res = run_bass_kernel_spmd(nc, in_maps, core_ids=list(range(NCORES)), trace=TRACE)
    LAST_RESULTS = res

    outs = [res.results[c]["out"] for c in range(NCORES)]
    if block_tokens is None:
        return np.concatenate(outs, axis=0)
    full = np.zeros((N_TOK, D_OUT), dtype=np.float32)
    flat_tok = block_tokens.ravel()
    flat_out = np.concatenate(outs, axis=0)
    valid = flat_tok >= 0
    full[flat_tok[valid]] = flat_out[valid]
    return full


def kernel(x, weight, num_inputs_per_group):
    x = np.ascontiguousarray(np.asarray(x, dtype=np.float32))
    weight = np.ascontiguousarray(np.asarray(weight, dtype=np.float32))
    num = np.asarray(num_inputs_per_group)
    seg = _seg_from_groups(num)
    if np.all(np.asarray(num, dtype=np.int64) == N_TOK // N_EXP):
        return _kernel_strassen(x, weight)
    return _kernel_general(x, weight, seg)
